# revision 30
# baseline (speedup 1.0000x reference)
"""Multi-head attention (B=2, S=2048, D=1024, H=16) on 8 Trainium2 cores.

Sharding: core c handles batch b = c//4 and head group g = c%4 (4 heads,
256 of the 1024 QKV output columns).

Pipeline (per core): the kernel is ACT(exp)-bound in steady state
(128 Exp instructions of [128,1024] ~ 1.1us each), so everything else is
scheduled around keeping ACT busy from ~17us on:

  1. DMA priority order: biases, wk, wq, xT (st-major, 4 chunks of 512
     s-cols so early chunks unblock early work), keepT in fine chunks
     interleaved, wv after the first xT chunk.
  2. Minimal pre-attention projections: kT chunk nn0 (sk tiles 0-3) and
     qT chunk for j=0, then v tiles 0-3. All remaining projection work
     (k0 nn1-3 emitted just before attention; q0 rest, all of k1/q1,
     v4-15) runs as "filler units" inside attention-phase PE slack.
  3. Attention per head pair in transposed layout: logitsT[sk, sq] via
     two concurrent K=64 row-group matmuls (2 heads), exp on ACT
     (scale folded), mask as multiplicative keepT (bf16 {0,1}; exact
     since exp(-1e9) underflows to 0), PV with ones-augmented V so the
     softmax denominator rides along as row 64.
  4. Tails: PE-transpose out_augT back to [s, dh] blocks, normalize by
     approx-reciprocal of the denominator row, DMA out per (j, head).

Matmuls in bf16 (host-cast), fp32 PSUM accumulation.
"""

import numpy as np

B, S, D, H = 2, 2048, 1024, 16
HD = D // H  # 64
HEADS_PER_CORE = 4
COLS = HEADS_PER_CORE * HD  # 256
N_CORES = 8
KT = D // 128  # 8 contraction tiles for projections
ST = S // 128  # 16 s tiles
SCALE = 1.0 / np.sqrt(np.float32(D))

_cache = {}


def _build_nc():
    import concourse.bass as bass
    import concourse.mybir as mybir
    import concourse.tile as tile
    from concourse.masks import make_identity

    f32 = mybir.dt.float32
    bf16 = mybir.dt.bfloat16

    nc = bass.Bass(trn_type="TRN2")

    xT = nc.dram_tensor("xT", [D, S], bf16, kind="ExternalInput")
    wq = nc.dram_tensor("wq", [D, COLS], bf16, kind="ExternalInput")
    wk = nc.dram_tensor("wk", [D, COLS], bf16, kind="ExternalInput")
    wv = nc.dram_tensor("wv", [D, COLS], bf16, kind="ExternalInput")
    bq = nc.dram_tensor("bq", [128, 2], f32, kind="ExternalInput")
    bk = nc.dram_tensor("bk", [128, 2], f32, kind="ExternalInput")
    bv = nc.dram_tensor("bv", [1, COLS], bf16, kind="ExternalInput")
    keepT = nc.dram_tensor("keepT", [S, S], bf16, kind="ExternalInput")
    o = nc.dram_tensor("o", [S, COLS], bf16, kind="ExternalOutput")

    with tile.TileContext(nc) as tc:
        with (
            tc.tile_pool(name="singles", bufs=1) as singles,
            tc.tile_pool(name="persist", bufs=1) as persist,
            tc.tile_pool(name="big_ps", bufs=2, space="PSUM") as big_ps,
            tc.tile_pool(name="pv_ps", bufs=2, space="PSUM") as pv_ps,
            tc.tile_pool(name="fil_ps", bufs=2, space="PSUM") as fil_ps,
            tc.tile_pool(name="expw", bufs=6) as expw_pool,
            tc.tile_pool(name="expw2", bufs=6) as expw2_pool,
            tc.tile_pool(name="tails", bufs=4) as tails,
        ):
            # ---- constants + tiny DMAs ----
            ones_col = singles.tile([1, 128], bf16)
            nc.vector.memset(ones_col, 1.0)
            ones_row = singles.tile([1, 512], bf16)
            nc.vector.memset(ones_row, 1.0)
            bq_sb = singles.tile([128, 2], f32)
            nc.sync.dma_start(out=bq_sb, in_=bq[:, :])
            bk_sb = singles.tile([128, 2], f32)
            nc.sync.dma_start(out=bk_sb, in_=bk[:, :])
            bv_sb = singles.tile([1, COLS], bf16)
            nc.sync.dma_start(out=bv_sb, in_=bv[:, :])

            # ---- persistent SBUF tensors ----
            wq_sb = persist.tile([128, KT, COLS], bf16)
            wk_sb = persist.tile([128, KT, COLS], bf16)
            wv_sb = persist.tile([128, KT, COLS], bf16)
            xT_sb = persist.tile([128, KT, S], bf16)
            keepT_sb = persist.tile([128, ST, S], bf16)
            qT_sb = persist.tile([128, 2, S], bf16)
            kT_sb = persist.tile([128, 2, S], bf16)
            v_aug = persist.tile([128, ST, HEADS_PER_CORE, HD + 1], bf16)
            nc.vector.memset(v_aug[:, :, :, HD : HD + 1], 1.0)

            # ---- PE warmup: dummy rank-1 accumulation so the HAM clock
            # gate opens (~3.4us of activity) while input DMAs stream ----
            warm_ps = big_ps.tile([128, 512], f32, tag="big", name="warm")
            for w in range(8):
                nc.tensor.matmul(
                    warm_ps,
                    lhsT=ones_col[:, :],
                    rhs=ones_row[:, :],
                    start=(w == 0),
                    stop=(w == 7),
                    skip_group_check=True,
                )

            # ---- bulk DMAs, emission order = sync-queue priority ----
            def dma_w(dst, src, b0, b1):  # col-block slices of a weight
                nc.sync.dma_start(
                    out=dst[:, :, 128 * b0 : 128 * b1],
                    in_=src[:, 128 * b0 : 128 * b1].rearrange(
                        "(kt p) c -> p kt c", p=128
                    ),
                )

            xT_r = xT[:, :].rearrange("kt (nst s) -> kt nst s", nst=4)
            keepT_r = keepT[:, :].rearrange("(i p) s -> p i s", p=128)

            def dma_xT(c):  # 512 s-cols: st tiles 4c..4c+3
                nc.sync.dma_start(
                    out=xT_sb[:, :, 512 * c : 512 * (c + 1)],
                    in_=xT_r[:, c, :].rearrange("(kt p) s -> p kt s", p=128),
                )

            def dma_keepT(i0, i1):
                nc.sync.dma_start(
                    out=keepT_sb[:, i0:i1, :], in_=keepT_r[:, i0:i1, :]
                )

            # Two parallel DMA queues: sync carries xT (gates projections)
            # then wv/keepT; the idle-until-attention ACT HWDGE queue carries
            # weights + early/late keepT chunks.
            def dma_w_act(dst, src, b0, b1):
                nc.scalar.dma_start(
                    out=dst[:, :, 128 * b0 : 128 * b1],
                    in_=src[:, 128 * b0 : 128 * b1].rearrange(
                        "(kt p) c -> p kt c", p=128
                    ),
                )

            dma_w_act(wk_sb, wk, 0, 1)
            dma_w_act(wq_sb, wq, 0, 1)
            nc.scalar.dma_start(
                out=keepT_sb[:, 0:2, :], in_=keepT_r[:, 0:2, :]
            )
            dma_w_act(wk_sb, wk, 1, 2)
            dma_w_act(wq_sb, wq, 1, 2)
            nc.scalar.dma_start(
                out=keepT_sb[:, 8:12, :], in_=keepT_r[:, 8:12, :]
            )
            nc.scalar.dma_start(
                out=keepT_sb[:, 12:16, :], in_=keepT_r[:, 12:16, :]
            )

            dma_xT(0)
            dma_xT(1)
            dma_keepT(2, 4)
            nc.sync.dma_start(
                out=wv_sb, in_=wv[:, :].rearrange("(kt p) c -> p kt c", p=128)
            )
            dma_xT(2)
            dma_keepT(4, 6)
            dma_xT(3)
            dma_keepT(6, 8)

            # ---- projection units ----
            def qk_halfgroup(which, blk, jh, nn, pool, tag="fill"):
                w_sb, b_sb, dst = (
                    (wq_sb, bq_sb, qT_sb),
                    (wk_sb, bk_sb, kT_sb),
                )[which]
                ps = pool.tile([128, 512], f32, tag=tag, name="qkhg")
                for kt in range(KT):
                    nc.tensor.matmul(
                        ps,
                        lhsT=w_sb[:, kt, blk * 128 : (blk + 1) * 128],
                        rhs=xT_sb[
                            :, kt, jh * 1024 + nn * 512 : jh * 1024 + (nn + 1) * 512
                        ],
                        start=(kt == 0),
                        stop=(kt == KT - 1),
                        skip_group_check=True,
                    )
                nc.vector.tensor_scalar_add(
                    out=dst[
                        :, blk, jh * 1024 + nn * 512 : jh * 1024 + (nn + 1) * 512
                    ],
                    in0=ps,
                    scalar1=b_sb[:, blk : blk + 1],
                )

            def vproj(st, pool, tag="fill"):
                psv = pool.tile([128, COLS], f32, tag=tag, name="vproj")
                nc.tensor.matmul(
                    psv,
                    lhsT=ones_col[:, :],
                    rhs=bv_sb[:, :],
                    start=True,
                    stop=False,
                    skip_group_check=True,
                )
                for kt in range(KT):
                    nc.tensor.matmul(
                        psv,
                        lhsT=xT_sb[:, kt, st * 128 : (st + 1) * 128],
                        rhs=wv_sb[:, kt, :],
                        start=False,
                        stop=(kt == KT - 1),
                        skip_group_check=True,
                    )
                nc.vector.tensor_copy(
                    out=v_aug[:, st, :, 0:HD],
                    in_=psv.rearrange("p (h d) -> p h d", h=HEADS_PER_CORE),
                )

            # ---- pre-attention minimal set (everything else is fillers) ----
            # NOTE: the tile framework dataflow is emission-ordered — any
            # producer (projection) MUST be emitted before its first consumer.
            qk_halfgroup(1, 0, 0, 0, big_ps, tag="big")  # kT sk tiles 0-3
            qk_halfgroup(0, 0, 0, 0, big_ps, tag="big")  # qT for j=0
            vproj(0, fil_ps)  # consumed by PV at j0 tile 0

            # ---- filler schedule: (hp, j, i) -> [unit fns] ----
            # Emission slot ~= scheduler priority; deps (xT chunk arrival,
            # pool slots) pace actual execution. No fillers at i<1 of a j.
            fill_at = {}

            def at(hp, j, i, fn):
                fill_at.setdefault((hp, j, i), []).append(fn)

            # k0 rest: sk tiles 4-7 / 8-11 / 12-15 (deadlines j0 tiles 4/8/12)
            at(0, 0, 1, lambda: qk_halfgroup(1, 0, 0, 1, fil_ps))
            at(0, 0, 3, lambda: qk_halfgroup(1, 0, 1, 0, fil_ps))
            at(0, 0, 7, lambda: qk_halfgroup(1, 0, 1, 1, fil_ps))
            # v1-15: v(st) needed at j0 tile st (PV may lag a few tiles);
            # slot <= st so the write is emitted before the consuming PV
            for st in range(1, ST):
                at(0, 0, max(1, st - 2), lambda st=st: vproj(st, fil_ps))
            # q0 chunks for j1..j3
            at(0, 0, 12, lambda: qk_halfgroup(0, 0, 0, 1, fil_ps))
            at(0, 1, 2, lambda: qk_halfgroup(0, 0, 1, 0, fil_ps))
            at(0, 2, 2, lambda: qk_halfgroup(0, 0, 1, 1, fil_ps))
            # k1 + q1-j0 done well before pair1; j3 is kept filler-free so
            # its PE slack absorbs any backlog before the pair switch
            at(0, 1, 5, lambda: qk_halfgroup(1, 1, 0, 0, fil_ps))
            at(0, 1, 10, lambda: qk_halfgroup(1, 1, 0, 1, fil_ps))
            at(0, 2, 5, lambda: qk_halfgroup(1, 1, 1, 0, fil_ps))
            at(0, 2, 10, lambda: qk_halfgroup(1, 1, 1, 1, fil_ps))
            at(0, 2, 13, lambda: qk_halfgroup(0, 1, 0, 0, fil_ps))
            # q1 chunks for pair1 j1..j3
            at(1, 0, 4, lambda: qk_halfgroup(0, 1, 0, 1, fil_ps))
            at(1, 1, 4, lambda: qk_halfgroup(0, 1, 1, 0, fil_ps))
            at(1, 2, 4, lambda: qk_halfgroup(0, 1, 1, 1, fil_ps))

            # ---- attention ----
            # Tails are software-pipelined: the tail of block j is emitted
            # two tiles into block j+1 so the scheduler prioritizes the next
            # block's logits (which feed ACT) over tail transposes.
            pending_tails = []

            def attention_pair(hp):
                blk = hp
                for j in range(4):
                    pvs = [
                        pv_ps.tile([HD + 1, 512], f32, tag="pv", name=f"pv{e}")
                        for e in range(2)
                    ]
                    for i in range(ST):
                        if i == 2:
                            for fn in pending_tails:
                                fn()
                            pending_tails.clear()
                        for fn in fill_at.pop((hp, j, i), ()):
                            fn()
                        lgp = big_ps.tile([128, 1024], f32, tag="big")
                        for e in range(2):
                            po = e * 64
                            nc.tensor.matmul(
                                lgp[:, e * 512 : (e + 1) * 512],
                                lhsT=kT_sb[
                                    po : po + 64, blk, i * 128 : (i + 1) * 128
                                ],
                                rhs=qT_sb[
                                    po : po + 64, blk, j * 512 : (j + 1) * 512
                                ],
                                start=True,
                                stop=True,
                                skip_group_check=True,
                            )
                        ex = expw_pool.tile([128, 1024], bf16)
                        nc.scalar.activation(
                            out=ex,
                            in_=lgp,
                            func=mybir.ActivationFunctionType.Exp,
                            scale=float(SCALE),
                        )
                        # mask both heads' halves with the same keepT slice,
                        # read twice via a stride-0 broadcast dim
                        ex2 = expw2_pool.tile([128, 1024], bf16)
                        k_ap = keepT_sb[:, i, j * 512 : (j + 1) * 512]
                        k_bcast = bass.AP(
                            tensor=k_ap.tensor,
                            offset=k_ap.offset,
                            ap=[k_ap.ap[0], [0, 2], *k_ap.ap[1:]],
                        )
                        # During pair0 j0/j1 the DVE is saturated (masks +
                        # projection evictions); offload some mask muls to
                        # the otherwise-idle GPSIMD (SBUF-only operands).
                        mask_eng = (
                            nc.gpsimd
                            if hp == 0 and j < 2 and i % 3 == 2
                            else nc.vector
                        )
                        mask_eng.tensor_mul(
                            out=ex2.rearrange("p (e n) -> p e n", e=2),
                            in0=ex.rearrange("p (e n) -> p e n", e=2),
                            in1=k_bcast,
                        )
                        for e in range(2):
                            nc.tensor.matmul(
                                pvs[e],
                                lhsT=v_aug[:, i, 2 * hp + e, :],
                                rhs=ex2[:, e * 512 : (e + 1) * 512],
                                start=(i == 0),
                                stop=(i == ST - 1),
                                skip_group_check=True,
                            )
                    # tail: evict both heads to bf16 now (frees pv slots);
                    # the transpose runs on the DMA XBAR (no PE, no PSUM) and
                    # is deferred into the next block. pv_sb is padded to 80
                    # partitions for XBAR 16-row alignment; cols 65-79 of the
                    # transposed tile are junk and never read.
                    pv_sbs = []
                    for e in range(2):
                        pv_sb = tails.tile(
                            [80, 512], bf16, tag="pvsb", name=f"pv_sb{e}"
                        )
                        nc.vector.tensor_copy(out=pv_sb[0 : HD + 1, :], in_=pvs[e])
                        pv_sbs.append(pv_sb)

                    def tail(hp=hp, j=j, pv_sbs=pv_sbs, final=False):
                        for e in range(2):
                            h = 2 * hp + e
                            # final tail: spread the two heads across both
                            # HWDGE queues (ACT is idle after the last exp)
                            eng = nc.scalar if final and e == 0 else nc.sync
                            pv_sb = pv_sbs[e]
                            trs = tails.tile([128, 4, 80], bf16, tag="trs")
                            eng.dma_start_transpose(out=trs, in_=pv_sb)
                            rc = tails.tile([128, 4], f32, tag="rc")
                            nc.vector.reciprocal(
                                out=rc, in_=trs[:, :, HD : HD + 1]
                            )
                            ob = tails.tile([128, 4, HD], bf16, tag="ob")
                            for c in range(4):
                                nc.vector.tensor_scalar_mul(
                                    out=ob[:, c, :],
                                    in0=trs[:, c, 0:HD],
                                    scalar1=rc[:, c : c + 1],
                                )
                            eng.dma_start(
                                out=o[
                                    j * 512 : (j + 1) * 512, h * HD : (h + 1) * HD
                                ].rearrange("(c p) d -> p c d", p=128),
                                in_=ob,
                            )

                    pending_tails.append(tail)

            attention_pair(0)
            attention_pair(1)
            for fn in pending_tails:
                fn(final=True)
            pending_tails.clear()
            assert not fill_at, f"unconsumed fillers: {list(fill_at)}"

    # Workaround: this container's walrus encodes at most one sync wait per
    # instruction — split multi-wait instructions into single-wait NoOps.
    _split_multiwait(nc)
    return nc


def _split_multiwait(nc, max_waits: int = 1):
    import concourse.mybir as mybir

    for f in nc.m.functions:
        for blk in f.blocks:
            out = []
            changed = False
            for inst in blk.instructions:
                si = inst.sync_info
                if si is not None and len(si.on_wait) > max_waits:
                    waits = list(si.on_wait)
                    extra = waits[: len(waits) - max_waits]
                    keep = waits[len(waits) - max_waits :]
                    for k, w in enumerate(extra):
                        out.append(
                            mybir.InstNoOp(
                                name=f"{inst.name}-wfx{k}",
                                engine=inst.engine,
                                sync_info=mybir.SyncInfo(on_wait=[w], on_update=[]),
                                bass_nofuse=True,
                            )
                        )
                    inst.sync_info = mybir.SyncInfo(
                        on_wait=keep, on_update=list(si.on_update)
                    )
                    changed = True
                out.append(inst)
            if changed:
                blk.instructions = out


def _prep_in_maps(x, mask, Wq, bq, Wk, bk, Wv, bv):
    import ml_dtypes

    bf16 = ml_dtypes.bfloat16
    x = np.asarray(x, np.float32)
    mask = np.asarray(mask, bool)

    xT_b = [np.ascontiguousarray(x[b].T).astype(bf16) for b in range(B)]
    keepT_b = [
        np.ascontiguousarray((~mask[b, 0]).T).astype(bf16) for b in range(B)
    ]
    WqT = np.asarray(Wq, np.float32).T.astype(bf16)
    WkT = np.asarray(Wk, np.float32).T.astype(bf16)
    WvT = np.asarray(Wv, np.float32).T.astype(bf16)
    bq32 = np.asarray(bq, np.float32)
    bk32 = np.asarray(bk, np.float32)
    bv = np.asarray(bv, np.float32).astype(bf16)

    in_maps = []
    for c in range(N_CORES):
        b, g = divmod(c, 4)
        cols = slice(g * COLS, (g + 1) * COLS)
        in_maps.append(
            {
                "xT": xT_b[b],
                "wq": np.ascontiguousarray(WqT[:, cols]),
                "wk": np.ascontiguousarray(WkT[:, cols]),
                "wv": np.ascontiguousarray(WvT[:, cols]),
                "bq": np.ascontiguousarray(bq32[cols].reshape(2, 128).T),
                "bk": np.ascontiguousarray(bk32[cols].reshape(2, 128).T),
                "bv": np.ascontiguousarray(bv[cols].reshape(1, COLS)),
                "keepT": keepT_b[b],
            }
        )
    return in_maps


def kernel(x, mask, Wq, bq, Wk, bk, Wv, bv, _trace=False):
    from concourse.bass_utils import run_bass_kernel_spmd

    if "nc" not in _cache:
        _cache["nc"] = _build_nc()
    nc = _cache["nc"]

    in_maps = _prep_in_maps(x, mask, Wq, bq, Wk, bk, Wv, bv)
    res = run_bass_kernel_spmd(
        nc, in_maps, core_ids=list(range(N_CORES)), trace=_trace
    )
    _cache["last_result"] = res

    out = np.empty((B, S, D), np.float32)
    for c in range(N_CORES):
        b, g = divmod(c, 4)
        out[b, :, g * COLS : (g + 1) * COLS] = np.asarray(
            res.results[c]["o"], dtype=np.float32
        )
    return out


# revision 32
# speedup vs baseline: 1.0783x; 1.0783x over previous
"""Multi-head attention (B=2, S=2048, D=1024, H=16) on 8 Trainium2 cores.

Sharding: core c handles batch b = c//4 and head group g = c%4 (4 heads,
256 of the 1024 QKV output columns).

Pipeline (per core): the kernel is ACT(exp)-bound in steady state
(128 Exp instructions of [128,1024] ~ 1.1us each), so everything else is
scheduled around keeping ACT busy from ~17us on:

  1. DMA priority order: biases, wk, wq, xT (st-major, 4 chunks of 512
     s-cols so early chunks unblock early work), keepT in fine chunks
     interleaved, wv after the first xT chunk.
  2. Minimal pre-attention projections: kT chunk nn0 (sk tiles 0-3) and
     qT chunk for j=0, then v tiles 0-3. All remaining projection work
     (k0 nn1-3 emitted just before attention; q0 rest, all of k1/q1,
     v4-15) runs as "filler units" inside attention-phase PE slack.
  3. Attention per head pair in transposed layout: logitsT[sk, sq] via
     two concurrent K=64 row-group matmuls (2 heads), exp on ACT
     (scale folded), mask as multiplicative keepT (bf16 {0,1}; exact
     since exp(-1e9) underflows to 0), PV with ones-augmented V so the
     softmax denominator rides along as row 64.
  4. Tails: PE-transpose out_augT back to [s, dh] blocks, normalize by
     approx-reciprocal of the denominator row, DMA out per (j, head).

Matmuls in bf16 (host-cast), fp32 PSUM accumulation.
"""

import numpy as np

B, S, D, H = 2, 2048, 1024, 16
HD = D // H  # 64
HEADS_PER_CORE = 4
COLS = HEADS_PER_CORE * HD  # 256
N_CORES = 8
KT = D // 128  # 8 contraction tiles for projections
ST = S // 128  # 16 s tiles
SCALE = 1.0 / np.sqrt(np.float32(D))

_cache = {}


def _build_nc():
    import concourse.bass as bass
    import concourse.mybir as mybir
    import concourse.tile as tile
    from concourse.masks import make_identity

    f32 = mybir.dt.float32
    bf16 = mybir.dt.bfloat16

    nc = bass.Bass(trn_type="TRN2")

    xT = nc.dram_tensor("xT", [D, S], bf16, kind="ExternalInput")
    wq = nc.dram_tensor("wq", [D, COLS], bf16, kind="ExternalInput")
    wk = nc.dram_tensor("wk", [D, COLS], bf16, kind="ExternalInput")
    wv = nc.dram_tensor("wv", [D, COLS], bf16, kind="ExternalInput")
    bq = nc.dram_tensor("bq", [128, 2], f32, kind="ExternalInput")
    bk = nc.dram_tensor("bk", [128, 2], f32, kind="ExternalInput")
    bv = nc.dram_tensor("bv", [1, COLS], bf16, kind="ExternalInput")
    keepT = nc.dram_tensor("keepT", [S, S], bf16, kind="ExternalInput")
    o = nc.dram_tensor("o", [S, COLS], bf16, kind="ExternalOutput")

    with tile.TileContext(nc) as tc:
        with (
            tc.tile_pool(name="singles", bufs=1) as singles,
            tc.tile_pool(name="persist", bufs=1) as persist,
            tc.tile_pool(name="big_ps", bufs=2, space="PSUM") as big_ps,
            tc.tile_pool(name="pv_ps", bufs=2, space="PSUM") as pv_ps,
            tc.tile_pool(name="fil_ps", bufs=2, space="PSUM") as fil_ps,
            tc.tile_pool(name="expw", bufs=8) as expw_pool,
            tc.tile_pool(name="expw2", bufs=8) as expw2_pool,
            tc.tile_pool(name="tails", bufs=4) as tails,
        ):
            # ---- constants + tiny DMAs ----
            ones_col = singles.tile([1, 128], bf16)
            nc.vector.memset(ones_col, 1.0)
            ones_row = singles.tile([1, 512], bf16)
            nc.vector.memset(ones_row, 1.0)
            bq_sb = singles.tile([128, 2], f32)
            nc.sync.dma_start(out=bq_sb, in_=bq[:, :])
            bk_sb = singles.tile([128, 2], f32)
            nc.sync.dma_start(out=bk_sb, in_=bk[:, :])
            bv_sb = singles.tile([1, COLS], bf16)
            nc.sync.dma_start(out=bv_sb, in_=bv[:, :])

            # ---- persistent SBUF tensors ----
            wq_sb = persist.tile([128, KT, COLS], bf16)
            wk_sb = persist.tile([128, KT, COLS], bf16)
            wv_sb = persist.tile([128, KT, COLS], bf16)
            xT_sb = persist.tile([128, KT, S], bf16)
            keepT_sb = persist.tile([128, ST, S], bf16)
            qT_sb = persist.tile([128, 2, S], bf16)
            kT_sb = persist.tile([128, 2, S], bf16)
            v_aug = persist.tile([128, ST, HEADS_PER_CORE, HD + 1], bf16)
            nc.vector.memset(v_aug[:, :, :, HD : HD + 1], 1.0)

            # ---- PE warmup: dummy rank-1 accumulation so the HAM clock
            # gate opens (~3.4us of activity) while input DMAs stream ----
            warm_ps = big_ps.tile([128, 512], f32, tag="big", name="warm")
            for w in range(8):
                nc.tensor.matmul(
                    warm_ps,
                    lhsT=ones_col[:, :],
                    rhs=ones_row[:, :],
                    start=(w == 0),
                    stop=(w == 7),
                    skip_group_check=True,
                )

            # ---- bulk DMAs, emission order = sync-queue priority ----
            def dma_w(dst, src, b0, b1):  # col-block slices of a weight
                nc.sync.dma_start(
                    out=dst[:, :, 128 * b0 : 128 * b1],
                    in_=src[:, 128 * b0 : 128 * b1].rearrange(
                        "(kt p) c -> p kt c", p=128
                    ),
                )

            xT_r = xT[:, :].rearrange("kt (nst s) -> kt nst s", nst=4)
            keepT_r = keepT[:, :].rearrange("(i p) s -> p i s", p=128)

            def dma_xT(c):  # 512 s-cols: st tiles 4c..4c+3
                nc.sync.dma_start(
                    out=xT_sb[:, :, 512 * c : 512 * (c + 1)],
                    in_=xT_r[:, c, :].rearrange("(kt p) s -> p kt s", p=128),
                )

            def dma_keepT(i0, i1):
                nc.sync.dma_start(
                    out=keepT_sb[:, i0:i1, :], in_=keepT_r[:, i0:i1, :]
                )

            # Two parallel DMA queues: sync carries xT (gates projections)
            # then wv/keepT; the idle-until-attention ACT HWDGE queue carries
            # weights + early/late keepT chunks.
            def dma_w_act(dst, src, b0, b1):
                nc.scalar.dma_start(
                    out=dst[:, :, 128 * b0 : 128 * b1],
                    in_=src[:, 128 * b0 : 128 * b1].rearrange(
                        "(kt p) c -> p kt c", p=128
                    ),
                )

            # The scalar HWDGE queue is slow (~100 B/ns) — give it only the
            # two small weight blocks that gate the first projections.
            dma_w_act(wk_sb, wk, 0, 1)
            dma_w_act(wq_sb, wq, 0, 1)

            dma_xT(0)
            dma_keepT(0, 2)
            dma_xT(1)
            nc.sync.dma_start(
                out=wv_sb, in_=wv[:, :].rearrange("(kt p) c -> p kt c", p=128)
            )
            dma_xT(2)
            dma_keepT(2, 4)
            dma_xT(3)
            dma_keepT(4, 6)
            dma_keepT(6, 8)
            dma_w(wk_sb, wk, 1, 2)
            dma_w(wq_sb, wq, 1, 2)
            dma_keepT(8, 12)
            dma_keepT(12, 16)

            # ---- projection units ----
            def qk_halfgroup(which, blk, jh, nn, pool, tag="fill"):
                w_sb, b_sb, dst = (
                    (wq_sb, bq_sb, qT_sb),
                    (wk_sb, bk_sb, kT_sb),
                )[which]
                ps = pool.tile([128, 512], f32, tag=tag, name="qkhg")
                for kt in range(KT):
                    nc.tensor.matmul(
                        ps,
                        lhsT=w_sb[:, kt, blk * 128 : (blk + 1) * 128],
                        rhs=xT_sb[
                            :, kt, jh * 1024 + nn * 512 : jh * 1024 + (nn + 1) * 512
                        ],
                        start=(kt == 0),
                        stop=(kt == KT - 1),
                        skip_group_check=True,
                    )
                nc.vector.tensor_scalar_add(
                    out=dst[
                        :, blk, jh * 1024 + nn * 512 : jh * 1024 + (nn + 1) * 512
                    ],
                    in0=ps,
                    scalar1=b_sb[:, blk : blk + 1],
                )

            def vproj(st, pool, tag="fill"):
                psv = pool.tile([128, COLS], f32, tag=tag, name="vproj")
                nc.tensor.matmul(
                    psv,
                    lhsT=ones_col[:, :],
                    rhs=bv_sb[:, :],
                    start=True,
                    stop=False,
                    skip_group_check=True,
                )
                for kt in range(KT):
                    nc.tensor.matmul(
                        psv,
                        lhsT=xT_sb[:, kt, st * 128 : (st + 1) * 128],
                        rhs=wv_sb[:, kt, :],
                        start=False,
                        stop=(kt == KT - 1),
                        skip_group_check=True,
                    )
                nc.vector.tensor_copy(
                    out=v_aug[:, st, :, 0:HD],
                    in_=psv.rearrange("p (h d) -> p h d", h=HEADS_PER_CORE),
                )

            # ---- pre-attention minimal set (everything else is fillers) ----
            # NOTE: the tile framework dataflow is emission-ordered — any
            # producer (projection) MUST be emitted before its first consumer.
            qk_halfgroup(1, 0, 0, 0, big_ps, tag="big")  # kT sk tiles 0-3
            qk_halfgroup(0, 0, 0, 0, big_ps, tag="big")  # qT for j=0
            vproj(0, fil_ps)  # consumed by PV at j0 tile 0

            # ---- filler schedule: (hp, j, i) -> [unit fns] ----
            # Emission slot ~= scheduler priority; deps (xT chunk arrival,
            # pool slots) pace actual execution. No fillers at i<1 of a j.
            fill_at = {}

            def at(hp, j, i, fn):
                fill_at.setdefault((hp, j, i), []).append(fn)

            # k0 rest: sk tiles 4-7 / 8-11 / 12-15 (deadlines j0 tiles 4/8/12)
            at(0, 0, 1, lambda: qk_halfgroup(1, 0, 0, 1, fil_ps))
            at(0, 0, 3, lambda: qk_halfgroup(1, 0, 1, 0, fil_ps))
            at(0, 0, 7, lambda: qk_halfgroup(1, 0, 1, 1, fil_ps))
            # v1-15: v(st) needed at j0 tile st (PV may lag a few tiles);
            # slot <= st so the write is emitted before the consuming PV
            for st in range(1, ST):
                at(0, 0, max(1, st - 2), lambda st=st: vproj(st, fil_ps))
            # q0 chunks for j1..j3
            at(0, 0, 12, lambda: qk_halfgroup(0, 0, 0, 1, fil_ps))
            at(0, 1, 2, lambda: qk_halfgroup(0, 0, 1, 0, fil_ps))
            at(0, 2, 2, lambda: qk_halfgroup(0, 0, 1, 1, fil_ps))
            # k1 + q1-j0 done well before pair1; j3 is kept filler-free so
            # its PE slack absorbs any backlog before the pair switch
            at(0, 1, 5, lambda: qk_halfgroup(1, 1, 0, 0, fil_ps))
            at(0, 1, 10, lambda: qk_halfgroup(1, 1, 0, 1, fil_ps))
            at(0, 2, 5, lambda: qk_halfgroup(1, 1, 1, 0, fil_ps))
            at(0, 2, 10, lambda: qk_halfgroup(1, 1, 1, 1, fil_ps))
            at(0, 2, 13, lambda: qk_halfgroup(0, 1, 0, 0, fil_ps))
            # q1 chunks for pair1 j1..j3
            at(1, 0, 4, lambda: qk_halfgroup(0, 1, 0, 1, fil_ps))
            at(1, 1, 4, lambda: qk_halfgroup(0, 1, 1, 0, fil_ps))
            at(1, 2, 4, lambda: qk_halfgroup(0, 1, 1, 1, fil_ps))

            # ---- attention ----
            # Tails are software-pipelined: the tail of block j is emitted
            # two tiles into block j+1 so the scheduler prioritizes the next
            # block's logits (which feed ACT) over tail transposes.
            pending_tails = []

            def attention_pair(hp):
                blk = hp
                for j in range(4):
                    pvs = [
                        pv_ps.tile([HD + 1, 512], f32, tag="pv", name=f"pv{e}")
                        for e in range(2)
                    ]
                    for i in range(ST):
                        if i == 2:
                            for fn in pending_tails:
                                fn()
                            pending_tails.clear()
                        for fn in fill_at.pop((hp, j, i), ()):
                            fn()
                        lgp = big_ps.tile([128, 1024], f32, tag="big")
                        for e in range(2):
                            po = e * 64
                            nc.tensor.matmul(
                                lgp[:, e * 512 : (e + 1) * 512],
                                lhsT=kT_sb[
                                    po : po + 64, blk, i * 128 : (i + 1) * 128
                                ],
                                rhs=qT_sb[
                                    po : po + 64, blk, j * 512 : (j + 1) * 512
                                ],
                                start=True,
                                stop=True,
                                skip_group_check=True,
                            )
                        ex = expw_pool.tile([128, 1024], bf16)
                        nc.scalar.activation(
                            out=ex,
                            in_=lgp,
                            func=mybir.ActivationFunctionType.Exp,
                            scale=float(SCALE),
                        )
                        # mask both heads' halves with the same keepT slice,
                        # read twice via a stride-0 broadcast dim
                        ex2 = expw2_pool.tile([128, 1024], bf16)
                        k_ap = keepT_sb[:, i, j * 512 : (j + 1) * 512]
                        k_bcast = bass.AP(
                            tensor=k_ap.tensor,
                            offset=k_ap.offset,
                            ap=[k_ap.ap[0], [0, 2], *k_ap.ap[1:]],
                        )
                        # During pair0 j0/j1 the DVE is saturated (masks +
                        # projection evictions); offload some mask muls to
                        # the otherwise-idle GPSIMD (SBUF-only operands).
                        mask_eng = (
                            nc.gpsimd
                            if hp == 0 and j < 2 and i % 3 == 2
                            else nc.vector
                        )
                        mask_eng.tensor_mul(
                            out=ex2.rearrange("p (e n) -> p e n", e=2),
                            in0=ex.rearrange("p (e n) -> p e n", e=2),
                            in1=k_bcast,
                        )
                        for e in range(2):
                            nc.tensor.matmul(
                                pvs[e],
                                lhsT=v_aug[:, i, 2 * hp + e, :],
                                rhs=ex2[:, e * 512 : (e + 1) * 512],
                                start=(i == 0),
                                stop=(i == ST - 1),
                                skip_group_check=True,
                            )
                    # tail: evict both heads to bf16 now (frees pv slots);
                    # the transpose runs on the DMA XBAR (no PE, no PSUM) and
                    # is deferred into the next block. pv_sb is padded to 80
                    # partitions for XBAR 16-row alignment; cols 65-79 of the
                    # transposed tile are junk and never read.
                    pv_sbs = []
                    for e in range(2):
                        pv_sb = tails.tile(
                            [80, 512], bf16, tag="pvsb", name=f"pv_sb{e}"
                        )
                        nc.vector.tensor_copy(out=pv_sb[0 : HD + 1, :], in_=pvs[e])
                        pv_sbs.append(pv_sb)

                    def tail(hp=hp, j=j, pv_sbs=pv_sbs, final=False):
                        for e in range(2):
                            h = 2 * hp + e
                            # final tail: spread the two heads across both
                            # HWDGE queues (ACT is idle after the last exp)
                            eng = nc.scalar if final and e == 0 else nc.sync
                            pv_sb = pv_sbs[e]
                            trs = tails.tile([128, 4, 80], bf16, tag="trs")
                            eng.dma_start_transpose(out=trs, in_=pv_sb)
                            rc = tails.tile([128, 4], f32, tag="rc")
                            nc.vector.reciprocal(
                                out=rc, in_=trs[:, :, HD : HD + 1]
                            )
                            ob = tails.tile([128, 4, HD], bf16, tag="ob")
                            for c in range(4):
                                nc.vector.tensor_scalar_mul(
                                    out=ob[:, c, :],
                                    in0=trs[:, c, 0:HD],
                                    scalar1=rc[:, c : c + 1],
                                )
                            eng.dma_start(
                                out=o[
                                    j * 512 : (j + 1) * 512, h * HD : (h + 1) * HD
                                ].rearrange("(c p) d -> p c d", p=128),
                                in_=ob,
                            )

                    pending_tails.append(tail)

            attention_pair(0)
            attention_pair(1)
            for fn in pending_tails:
                fn(final=True)
            pending_tails.clear()
            assert not fill_at, f"unconsumed fillers: {list(fill_at)}"

    # Workaround: this container's walrus encodes at most one sync wait per
    # instruction — split multi-wait instructions into single-wait NoOps.
    _split_multiwait(nc)
    return nc


def _split_multiwait(nc, max_waits: int = 1):
    import concourse.mybir as mybir

    for f in nc.m.functions:
        for blk in f.blocks:
            out = []
            changed = False
            for inst in blk.instructions:
                si = inst.sync_info
                if si is not None and len(si.on_wait) > max_waits:
                    waits = list(si.on_wait)
                    extra = waits[: len(waits) - max_waits]
                    keep = waits[len(waits) - max_waits :]
                    for k, w in enumerate(extra):
                        out.append(
                            mybir.InstNoOp(
                                name=f"{inst.name}-wfx{k}",
                                engine=inst.engine,
                                sync_info=mybir.SyncInfo(on_wait=[w], on_update=[]),
                                bass_nofuse=True,
                            )
                        )
                    inst.sync_info = mybir.SyncInfo(
                        on_wait=keep, on_update=list(si.on_update)
                    )
                    changed = True
                out.append(inst)
            if changed:
                blk.instructions = out


def _prep_in_maps(x, mask, Wq, bq, Wk, bk, Wv, bv):
    import ml_dtypes

    bf16 = ml_dtypes.bfloat16
    x = np.asarray(x, np.float32)
    mask = np.asarray(mask, bool)

    xT_b = [np.ascontiguousarray(x[b].T).astype(bf16) for b in range(B)]
    keepT_b = [
        np.ascontiguousarray((~mask[b, 0]).T).astype(bf16) for b in range(B)
    ]
    WqT = np.asarray(Wq, np.float32).T.astype(bf16)
    WkT = np.asarray(Wk, np.float32).T.astype(bf16)
    WvT = np.asarray(Wv, np.float32).T.astype(bf16)
    bq32 = np.asarray(bq, np.float32)
    bk32 = np.asarray(bk, np.float32)
    bv = np.asarray(bv, np.float32).astype(bf16)

    in_maps = []
    for c in range(N_CORES):
        b, g = divmod(c, 4)
        cols = slice(g * COLS, (g + 1) * COLS)
        in_maps.append(
            {
                "xT": xT_b[b],
                "wq": np.ascontiguousarray(WqT[:, cols]),
                "wk": np.ascontiguousarray(WkT[:, cols]),
                "wv": np.ascontiguousarray(WvT[:, cols]),
                "bq": np.ascontiguousarray(bq32[cols].reshape(2, 128).T),
                "bk": np.ascontiguousarray(bk32[cols].reshape(2, 128).T),
                "bv": np.ascontiguousarray(bv[cols].reshape(1, COLS)),
                "keepT": keepT_b[b],
            }
        )
    return in_maps


def kernel(x, mask, Wq, bq, Wk, bk, Wv, bv, _trace=False):
    from concourse.bass_utils import run_bass_kernel_spmd

    if "nc" not in _cache:
        _cache["nc"] = _build_nc()
    nc = _cache["nc"]

    in_maps = _prep_in_maps(x, mask, Wq, bq, Wk, bk, Wv, bv)
    res = run_bass_kernel_spmd(
        nc, in_maps, core_ids=list(range(N_CORES)), trace=_trace
    )
    _cache["last_result"] = res

    out = np.empty((B, S, D), np.float32)
    for c in range(N_CORES):
        b, g = divmod(c, 4)
        out[b, :, g * COLS : (g + 1) * COLS] = np.asarray(
            res.results[c]["o"], dtype=np.float32
        )
    return out
